# revision 6
# baseline (speedup 1.0000x reference)
"""CPPN forward (12-layer tiny MLP over 4.2M pixels) on 8 TRN2 NeuronCores.

V2 strategy (pure data parallel; evolved from the fp32 baseline):
- Pixels sharded 8 ways; per core 524288 px padded to 208 supertiles (ST).
- One ST = 5 pixel-blocks x 512 px; 5 blocks block-diagonally packed into a
  116-row layout (85 id rows, 5 sin rows @64, 20 gauss rows @96; engine
  partition bases must be 32-aligned). Group = 4 STs -> free size F=2048.
- fp16 everywhere on SBUF (weights, hidden states): matmuls run at 1 cyc/row
  (vs 4 for fp32) and DVE elementwise ops get the 2x packed-16-bit mode.
  PSUM stays fp32, and the sine range reduction runs in fp32 for precision.
- Per layer per group:
    4 matmuls [K<=116, M=116, N=512] (fp16) -> P (PSUM fp32)
    copyB:  H = P + bias, split: DVE tensor_scalar_add on half the pixels,
            ACT Identity(+bias) on the other half (engine balance).
            bias = folded "-1" of the gaussian act 2*exp(-h^2)-1 of the
            previous layer (gauss rows x2 in the next weights).
    gauss:  SQ = H_g * H_g (DVE fp16 2x)  ->  H_g = Exp(-SQ) (ACT)
    sin:    SIN_MODE='direct': H_s = Sin2pi(P_s/2pi + b') - one ACT op,
            relies on the sin2pi table being accurate past +-0.55 turns.
            SIN_MODE='triple': s=Sin2pi(P_s/6pi + b'); H_s = s*(3-4s^2) via
            TT square + TSP(mult,add) + TT mult (DVE fp16 ops).
            (DVE has no fp mod op, so no exact range reduction.)
- Groups are processed in interleaved pairs (A/B software pipeline) so DVE,
  ACT and PE overlap across groups instead of serializing per layer.
- mybir has no Sin2pi enum, so activations are emitted as Sin and the
  serialized BIR JSON is patched Sin->Sin2pi before compilation (the
  exp_and_friends HW table holds {exp, square, identity, copy, sin2pi}).
"""
import sys, types
import numpy as np

sys.path.insert(0, "/opt/trn_rl_repo")

# ---------------------------------------------------------------- constants
N_PIX = 2048 * 2048
D_IN, D_HID, D_OUT = 4, 22, 3
N_HIDDEN = 11
N_CORES = 8
FD = 512                      # pixels per block (= matmul free dim)
BLOCKS = 5                    # blocks per supertile (5*22=110 partitions)
ST_PX = BLOCKS * FD           # 2560 px per supertile
GROUP = 4                     # supertiles per group (PSUM banks)
PX_CORE = N_PIX // N_CORES            # 524288
N_ST = -(-PX_CORE // ST_PX)           # 205
N_GROUP = -(-N_ST // GROUP)           # 52
N_ST_PAD = N_GROUP * GROUP            # 208
PX_PAD = N_ST_PAD * ST_PX             # 532480

ID_CH = list(range(15)) + [19, 20]    # 17 identity channels per block
GA_CH = [15, 16, 17, 18]
SI_CH = [21]
# engine partition bases must be 32-aligned -> layout:
#   rows 0..63   id channels 0..63
#   rows 64..68  sin (base 64)
#   rows 69..89  id channels 64..84
#   rows 90..95  zero pad
#   rows 96..115 gauss (base 96)
ROWS = 116
SIN0, GA0 = 64, 96
TWO_PI = 2.0 * np.pi
SIN_MODE = "triple"           # "direct" | "triple" (see probe results)

# ------------------------------------------------------------- host packing
def _row_of(b, c):
    """partition row of (block b, original channel c) in the ST layout"""
    if c in GA_CH:
        return GA0 + b * 4 + (c - 15)
    if c == 21:
        return SIN0 + b
    g = b * 17 + ID_CH.index(c)
    return g if g < 64 else 69 + (g - 64)

SIN_DIV = TWO_PI if SIN_MODE == "direct" else 3 * TWO_PI

def _out_scale(c):
    """scale on weight columns producing channel c's preactivation"""
    return 1.0 / SIN_DIV if c == 21 else 1.0

def _in_scale(c):
    """fold factor on weight rows consuming activation outputs"""
    return 2.0 if c in GA_CH else 1.0

def pack_weights(W_in, W_hidden, W_out):
    """Build the 13 block-diagonal lhsT matrices + bias vectors."""
    W_in, W_hidden, W_out = (np.asarray(W_in, np.float32),
                             np.asarray(W_hidden, np.float32),
                             np.asarray(W_out, np.float32))
    # MM1: x -> layer1 preact. lhsT [20, 116]
    lin = np.zeros((BLOCKS * 4, ROWS), np.float32)
    for b in range(BLOCKS):
        for ci in range(D_IN):
            for co in range(D_HID):
                lin[b * 4 + ci, _row_of(b, co)] = W_in[ci, co] * _out_scale(co)
    # MM2..12: hidden. lhsT [116, 116]
    lh = np.zeros((N_HIDDEN, ROWS, ROWS), np.float32)
    # bt cols 0..11: copyB bias of layer l (col 0 zero) on id+gauss rows.
    bt = np.zeros((ROWS, 12), np.float32)
    # bts cols 0..11: sin-row mod bias b/2pi + SIN_OFF.
    bts = np.zeros((5, 12), np.float32)
    for i in range(N_HIDDEN):
        W = W_hidden[i]
        for b in range(BLOCKS):
            for ci in range(D_HID):
                s = _in_scale(ci)
                ri = _row_of(b, ci)
                for co in range(D_HID):
                    lh[i, ri, _row_of(b, co)] = W[ci, co] * s * _out_scale(co)
        bvec = -W[15:19, :].sum(axis=0)        # folded -1 per output channel
        for b in range(BLOCKS):
            for co in range(D_HID):
                bt[_row_of(b, co), i + 1] = bvec[co]
            bts[b, i + 1] = bvec[21] / SIN_DIV
    # MM13: out. lhsT [116, 15] (+ obias on the packed [111] out layout)
    lo = np.zeros((ROWS, BLOCKS * 3), np.float32)
    for b in range(BLOCKS):
        for ci in range(D_HID):
            s = _in_scale(ci)
            for co in range(D_OUT):
                lo[_row_of(b, ci), b * 3 + co] = W_out[ci, co] * s
    bo = -W_out[15:19, :].sum(axis=0)          # [3]
    obias = np.zeros((111, 1), np.float32)
    for r in range(GROUP):
        for b in range(BLOCKS):
            for co in range(D_OUT):
                obias[32 * r + b * 3 + co, 0] = bo[co]
    return {"w_in": lin.astype(np.float16), "w_hid": lh.astype(np.float16),
            "w_out": lo.astype(np.float16), "bias": bt, "sbias": bts,
            "obias": obias}

def pack_x(x):
    """[N_PIX,4] -> per-core [52, 20, 4, 512] f16 arrays."""
    x = np.asarray(x, np.float32)
    out = []
    for k in range(N_CORES):
        shard = x[k * PX_CORE:(k + 1) * PX_CORE]
        pad = np.zeros((PX_PAD, D_IN), np.float32)
        pad[:PX_CORE] = shard
        a = pad.reshape(N_GROUP, GROUP, BLOCKS, FD, D_IN)
        a = a.transpose(0, 2, 4, 1, 3).reshape(N_GROUP, BLOCKS * D_IN, GROUP, FD)
        out.append(np.ascontiguousarray(a.astype(np.float16)))
    return out

_OUT_ROWS = np.array([[32 * r + b * 3 + co for b in range(BLOCKS) for co in range(D_OUT)]
                      for r in range(GROUP)])  # [4, 15]

def unpack_out(outs):
    """per-core [52, 111, 512] f16 -> [N_PIX, 3] f32"""
    full = np.empty((N_PIX, D_OUT), np.float32)
    for k, od in enumerate(outs):
        g = od.astype(np.float32)[:, _OUT_ROWS.reshape(-1), :]  # [52, 60, 512]
        g = g.reshape(N_GROUP, GROUP, BLOCKS, D_OUT, FD)
        g = g.transpose(0, 1, 2, 4, 3).reshape(PX_PAD, D_OUT)   # [532480, 3]
        full[k * PX_CORE:(k + 1) * PX_CORE] = g[:PX_CORE]
    return full

# ------------------------------------------------------------ device kernel
_CACHE = {}

def _shim_hooks():
    import antenv
    if "antenv.axon_hooks" in sys.modules:
        return
    hooks = types.ModuleType("antenv.axon_hooks")
    hooks._hook = None
    hooks.set_axon_ntff_profile_hook = lambda h: setattr(hooks, "_hook", h)
    hooks.get_axon_ntff_profile_hook = lambda: hooks._hook
    sys.modules["antenv.axon_hooks"] = hooks
    antenv.axon_hooks = hooks
    try:
        from trn_agent_boot.trn_boot import _ntff_profile_via_ctypes
        hooks._hook = _ntff_profile_via_ctypes("/opt/axon/libaxon_pjrt.so")
    except Exception:
        pass

def _build():
    _shim_hooks()
    import concourse.bacc as bacc_mod
    import concourse.mybir as mybir
    import concourse.tile as tile
    from concourse.hw_specs import get_activation_tables as _real_gat

    AFT = mybir.ActivationFunctionType
    ALU = mybir.AluOpType
    ours = {AFT.Square, AFT.Exp, AFT.Identity, AFT.Copy, AFT.Sin, AFT.Relu}

    def _doctored_gat(arch):
        tabs = dict(_real_gat(arch))
        return {n: (set(f) | ours if n == "exp_and_friends" else set(f) - ours)
                for n, f in tabs.items()}

    bacc_mod.get_activation_tables = _doctored_gat

    f32 = mybir.dt.float32
    f16 = mybir.dt.float16
    nc = bacc_mod.Bacc(None, target_bir_lowering=False, debug=False)
    x_d = nc.declare_dram_parameter("x", [N_GROUP, 20, GROUP, FD], f16, isOutput=False)
    win_d = nc.declare_dram_parameter("w_in", [20, ROWS], f16, isOutput=False)
    wh_d = nc.declare_dram_parameter("w_hid", [N_HIDDEN, ROWS, ROWS], f16, isOutput=False)
    wo_d = nc.declare_dram_parameter("w_out", [ROWS, 15], f16, isOutput=False)
    b_d = nc.declare_dram_parameter("bias", [ROWS, 12], f32, isOutput=False)
    bs_d = nc.declare_dram_parameter("sbias", [5, 12], f32, isOutput=False)
    ob_d = nc.declare_dram_parameter("obias", [111, 1], f32, isOutput=False)
    o_d = nc.declare_dram_parameter("out", [N_GROUP, 111, FD], f16, isOutput=True)

    with tile.TileContext(nc) as tc:
        with (tc.tile_pool(name="wpool", bufs=1) as wpool,
              tc.tile_pool(name="xpool", bufs=4) as xpool,
              tc.tile_pool(name="hpool", bufs=2) as hpool,
              tc.tile_pool(name="tpool", bufs=2) as tpool,
              tc.tile_pool(name="opool", bufs=3) as opool,
              tc.tile_pool(name="ppool", bufs=2, space="PSUM") as ppool):
            win = wpool.tile([20, ROWS], f16)
            wh = [wpool.tile([ROWS, ROWS], f16, tag=f"wh{i}", name=f"wh{i}")
                  for i in range(N_HIDDEN)]
            wo = wpool.tile([ROWS, 15], f16)
            bt = wpool.tile([ROWS, 12], f32)
            bs = wpool.tile([5, 12], f32)
            ob = wpool.tile([111, 1], f32)
            nc.sync.dma_start(out=win[:], in_=win_d[:])
            for i in range(N_HIDDEN):
                nc.sync.dma_start(out=wh[i][:], in_=wh_d[i])
            nc.sync.dma_start(out=wo[:], in_=wo_d[:])
            nc.sync.dma_start(out=bt[:], in_=b_d[:])
            nc.sync.dma_start(out=bs[:], in_=bs_d[:])
            nc.sync.dma_start(out=ob[:], in_=ob_d[:])

            HC = GROUP // 2                   # copyB split point (DVE|ACT)
            for pr in range(N_GROUP // 2):
                gpair = (2 * pr, 2 * pr + 1)
                xg = {}
                for par, g in enumerate(gpair):
                    xg[par] = xpool.tile([20, GROUP, FD], f16, tag="xg",
                                         name=f"xg{par}")
                    nc.sync.dma_start(out=xg[par][:], in_=x_d[g])
                hprev = {0: None, 1: None}
                for mm in range(1, 13):       # 12 hidden matmul rounds
                    for par in range(2):
                        P = ppool.tile([ROWS, GROUP, FD], f32, tag="pm")
                        for r in range(GROUP):
                            if mm == 1:
                                nc.tensor.matmul(P[:, r, :], win[:],
                                                 xg[par][:, r, :],
                                                 start=True, stop=True)
                            else:
                                nc.tensor.matmul(P[:, r, :], wh[mm - 2][:],
                                                 hprev[par][:, r, :],
                                                 start=True, stop=True)
                        H = hpool.tile([ROWS, GROUP, FD], f16, tag=f"H{par}")
                        if SIN_MODE == "triple":
                            # s1 = Sin2pi(P_s/6pi + b') straight from PSUM
                            s1 = tpool.tile([5, GROUP, FD], f16, tag=f"s1{par}")
                            nc.scalar.activation(s1[:], P[SIN0:SIN0 + 5, :, :],
                                                 AFT.Sin, bias=bs[:, mm - 1:mm],
                                                 scale=1.0)
                        # copyB: H = P + bias, split across DVE and ACT
                        nc.scalar.activation(H[:, HC:, :], P[:, HC:, :],
                                             AFT.Identity,
                                             bias=bt[:, mm - 1:mm], scale=1.0)
                        nc.vector.tensor_scalar_add(H[:, :HC, :], P[:, :HC, :],
                                                    bt[:, mm - 1:mm])
                        if SIN_MODE == "direct":
                            nc.scalar.activation(H[SIN0:SIN0 + 5, :, :],
                                                 P[SIN0:SIN0 + 5, :, :],
                                                 AFT.Sin, bias=bs[:, mm - 1:mm],
                                                 scale=1.0)
                        else:
                            # H_s = s1*(3-4*s1^2)  (triple angle)
                            s2 = tpool.tile([5, GROUP, FD], f16, tag=f"s2{par}")
                            nc.vector.tensor_tensor(s2[:], s1[:], s1[:],
                                                    op=ALU.mult)
                            q = tpool.tile([5, GROUP, FD], f16, tag=f"q{par}")
                            nc.vector.tensor_scalar(q[:], s2[:], -4.0, 3.0,
                                                    op0=ALU.mult, op1=ALU.add)
                            nc.vector.tensor_tensor(H[SIN0:SIN0 + 5, :, :],
                                                    q[:], s1[:], op=ALU.mult)
                        # gauss: H_g = exp(-(H_g)^2)
                        SQ = tpool.tile([20, GROUP, FD], f16, tag=f"sq{par}")
                        nc.vector.tensor_tensor(SQ[:], H[GA0:GA0 + 20, :, :],
                                                H[GA0:GA0 + 20, :, :],
                                                op=ALU.mult)
                        nc.scalar.activation(H[GA0:GA0 + 20, :, :], SQ[:],
                                             AFT.Exp, bias=0.0, scale=-1.0)
                        hprev[par] = H
                for par, g in enumerate(gpair):   # output round
                    O = ppool.tile([111, FD], f32, tag="pm")
                    for r in range(GROUP):
                        nc.tensor.matmul(O[32 * r:32 * r + 15, :], wo[:],
                                         hprev[par][:, r, :], start=True,
                                         stop=True, tile_position=(0, 32 * r))
                    ot = opool.tile([111, FD], f16, tag="ot")
                    nc.vector.tensor_scalar_add(ot[:], O[:], ob[:])
                    nc.sync.dma_start(out=o_d[g], in_=ot[:])
    nc.compile()

    _orig = nc.to_json_bytes
    nc.to_json_bytes = lambda: _orig().replace(b'"func":"Sin"', b'"func":"Sin2pi"')
    return nc

def _get_nc():
    if "nc" not in _CACHE:
        _CACHE["nc"] = _build()
    return _CACHE["nc"]

def make_in_maps(x_cores, w):
    return [{"x": x_cores[k], "w_in": w["w_in"], "w_hid": w["w_hid"],
             "w_out": w["w_out"], "bias": w["bias"], "sbias": w["sbias"],
             "obias": w["obias"]} for k in range(N_CORES)]

def run_device(x_cores, w):
    from concourse.bass_utils import run_bass_kernel_spmd
    nc = _get_nc()
    res = run_bass_kernel_spmd(nc, make_in_maps(x_cores, w),
                               list(range(N_CORES)), trace=False)
    return [res.results[k]["out"] for k in range(N_CORES)]

def kernel(x, W_in, W_hidden, W_out):
    w = pack_weights(W_in, W_hidden, W_out)
    x_cores = pack_x(x)
    outs = run_device(x_cores, w)
    return unpack_out(outs)


# revision 7
# speedup vs baseline: 1.1235x; 1.1235x over previous
"""CPPN forward (12-layer tiny MLP over 4.2M pixels) on 8 TRN2 NeuronCores.

V2 strategy (pure data parallel; evolved from the fp32 baseline):
- Pixels sharded 8 ways; per core 524288 px padded to 208 supertiles (ST).
- One ST = 5 pixel-blocks x 512 px; 5 blocks block-diagonally packed into a
  116-row layout (85 id rows, 5 sin rows @64, 20 gauss rows @96; engine
  partition bases must be 32-aligned). Group = 4 STs -> free size F=2048.
- fp16 everywhere on SBUF (weights, hidden states): matmuls run at 1 cyc/row
  (vs 4 for fp32) and DVE elementwise ops get the 2x packed-16-bit mode.
  PSUM stays fp32, and the sine range reduction runs in fp32 for precision.
- Per layer per group:
    4 matmuls [K<=116, M=116, N=512] (fp16) -> P (PSUM fp32)
    copyB:  H = P + bias, split: DVE tensor_scalar_add on half the pixels,
            ACT Identity(+bias) on the other half (engine balance).
            bias = folded "-1" of the gaussian act 2*exp(-h^2)-1 of the
            previous layer (gauss rows x2 in the next weights).
    gauss:  SQ = H_g * H_g (DVE fp16 2x)  ->  H_g = Exp(-SQ) (ACT)
    sin:    SIN_MODE='direct': H_s = Sin2pi(P_s/2pi + b') - one ACT op,
            relies on the sin2pi table being accurate past +-0.55 turns.
            SIN_MODE='triple': s=Sin2pi(P_s/6pi + b'); H_s = s*(3-4s^2) via
            TT square + TSP(mult,add) + TT mult (DVE fp16 ops).
            (DVE has no fp mod op, so no exact range reduction.)
- Groups are processed in interleaved pairs (A/B software pipeline) so DVE,
  ACT and PE overlap across groups instead of serializing per layer.
- mybir has no Sin2pi enum, so activations are emitted as Sin and the
  serialized BIR JSON is patched Sin->Sin2pi before compilation (the
  exp_and_friends HW table holds {exp, square, identity, copy, sin2pi}).
"""
import sys, types
import numpy as np

sys.path.insert(0, "/opt/trn_rl_repo")

# ---------------------------------------------------------------- constants
N_PIX = 2048 * 2048
D_IN, D_HID, D_OUT = 4, 22, 3
N_HIDDEN = 11
N_CORES = 8
FD = 512                      # pixels per block (= matmul free dim)
BLOCKS = 5                    # blocks per supertile (5*22=110 partitions)
ST_PX = BLOCKS * FD           # 2560 px per supertile
GROUP = 4                     # supertiles per group (PSUM banks)
PX_CORE = N_PIX // N_CORES            # 524288
N_ST = -(-PX_CORE // ST_PX)           # 205
N_GROUP = -(-N_ST // GROUP)           # 52
N_ST_PAD = N_GROUP * GROUP            # 208
PX_PAD = N_ST_PAD * ST_PX             # 532480

ID_CH = list(range(15)) + [19, 20]    # 17 identity channels per block
GA_CH = [15, 16, 17, 18]
SI_CH = [21]
# engine partition bases must be 32-aligned -> layout:
#   rows 0..63   id channels 0..63
#   rows 64..68  sin (base 64)
#   rows 69..89  id channels 64..84
#   rows 90..95  zero pad
#   rows 96..115 gauss (base 96)
ROWS = 116
SIN0, GA0 = 64, 96
TWO_PI = 2.0 * np.pi
SIN_MODE = "triple"           # "direct" | "triple" (see probe results)

# ------------------------------------------------------------- host packing
def _row_of(b, c):
    """partition row of (block b, original channel c) in the ST layout"""
    if c in GA_CH:
        return GA0 + b * 4 + (c - 15)
    if c == 21:
        return SIN0 + b
    g = b * 17 + ID_CH.index(c)
    return g if g < 64 else 69 + (g - 64)

SIN_DIV = TWO_PI if SIN_MODE == "direct" else 3 * TWO_PI

def _out_scale(c):
    """scale on weight columns producing channel c's preactivation"""
    return 1.0 / SIN_DIV if c == 21 else 1.0

def _in_scale(c):
    """fold factor on weight rows consuming activation outputs"""
    return 2.0 if c in GA_CH else 1.0

def pack_weights(W_in, W_hidden, W_out):
    """Build the 13 block-diagonal lhsT matrices + bias vectors."""
    W_in, W_hidden, W_out = (np.asarray(W_in, np.float32),
                             np.asarray(W_hidden, np.float32),
                             np.asarray(W_out, np.float32))
    # MM1: x -> layer1 preact. lhsT [20, 116]
    lin = np.zeros((BLOCKS * 4, ROWS), np.float32)
    for b in range(BLOCKS):
        for ci in range(D_IN):
            for co in range(D_HID):
                lin[b * 4 + ci, _row_of(b, co)] = W_in[ci, co] * _out_scale(co)
    # MM2..12: hidden. lhsT [116, 116]
    lh = np.zeros((N_HIDDEN, ROWS, ROWS), np.float32)
    # bt cols 0..11: copyB bias of layer l (col 0 zero) on id+gauss rows.
    bt = np.zeros((ROWS, 12), np.float32)
    # bts cols 0..11: sin-row mod bias b/2pi + SIN_OFF.
    bts = np.zeros((5, 12), np.float32)
    for i in range(N_HIDDEN):
        W = W_hidden[i]
        for b in range(BLOCKS):
            for ci in range(D_HID):
                s = _in_scale(ci)
                ri = _row_of(b, ci)
                for co in range(D_HID):
                    lh[i, ri, _row_of(b, co)] = W[ci, co] * s * _out_scale(co)
        bvec = -W[15:19, :].sum(axis=0)        # folded -1 per output channel
        for b in range(BLOCKS):
            for co in range(D_HID):
                bt[_row_of(b, co), i + 1] = bvec[co]
            bts[b, i + 1] = bvec[21] / SIN_DIV
    # MM13: out. lhsT [116, 15] (+ obias on the packed [111] out layout)
    lo = np.zeros((ROWS, BLOCKS * 3), np.float32)
    for b in range(BLOCKS):
        for ci in range(D_HID):
            s = _in_scale(ci)
            for co in range(D_OUT):
                lo[_row_of(b, ci), b * 3 + co] = W_out[ci, co] * s
    bo = -W_out[15:19, :].sum(axis=0)          # [3]
    obias = np.zeros((111, 1), np.float32)
    for r in range(GROUP):
        for b in range(BLOCKS):
            for co in range(D_OUT):
                obias[32 * r + b * 3 + co, 0] = bo[co]
    return {"w_in": lin.astype(np.float16), "w_hid": lh.astype(np.float16),
            "w_out": lo.astype(np.float16), "bias": bt, "sbias": bts,
            "obias": obias}

def pack_x(x):
    """[N_PIX,4] -> per-core [52, 20, 4, 512] f16 arrays."""
    x = np.asarray(x, np.float32)
    out = []
    for k in range(N_CORES):
        shard = x[k * PX_CORE:(k + 1) * PX_CORE]
        pad = np.zeros((PX_PAD, D_IN), np.float32)
        pad[:PX_CORE] = shard
        a = pad.reshape(N_GROUP, GROUP, BLOCKS, FD, D_IN)
        a = a.transpose(0, 2, 4, 1, 3).reshape(N_GROUP, BLOCKS * D_IN, GROUP, FD)
        out.append(np.ascontiguousarray(a.astype(np.float16)))
    return out

_OUT_ROWS = np.array([[32 * r + b * 3 + co for b in range(BLOCKS) for co in range(D_OUT)]
                      for r in range(GROUP)])  # [4, 15]

def unpack_out(outs):
    """per-core [52, 111, 512] f16 -> [N_PIX, 3] f32"""
    full = np.empty((N_PIX, D_OUT), np.float32)
    for k, od in enumerate(outs):
        g = od.astype(np.float32)[:, _OUT_ROWS.reshape(-1), :]  # [52, 60, 512]
        g = g.reshape(N_GROUP, GROUP, BLOCKS, D_OUT, FD)
        g = g.transpose(0, 1, 2, 4, 3).reshape(PX_PAD, D_OUT)   # [532480, 3]
        full[k * PX_CORE:(k + 1) * PX_CORE] = g[:PX_CORE]
    return full

# ------------------------------------------------------------ device kernel
_CACHE = {}

def _shim_hooks():
    import antenv
    if "antenv.axon_hooks" in sys.modules:
        return
    hooks = types.ModuleType("antenv.axon_hooks")
    hooks._hook = None
    hooks.set_axon_ntff_profile_hook = lambda h: setattr(hooks, "_hook", h)
    hooks.get_axon_ntff_profile_hook = lambda: hooks._hook
    sys.modules["antenv.axon_hooks"] = hooks
    antenv.axon_hooks = hooks
    try:
        from trn_agent_boot.trn_boot import _ntff_profile_via_ctypes
        hooks._hook = _ntff_profile_via_ctypes("/opt/axon/libaxon_pjrt.so")
    except Exception:
        pass

def _build():
    _shim_hooks()
    import concourse.bacc as bacc_mod
    import concourse.mybir as mybir
    import concourse.tile as tile
    from concourse.hw_specs import get_activation_tables as _real_gat

    AFT = mybir.ActivationFunctionType
    ALU = mybir.AluOpType
    ours = {AFT.Square, AFT.Exp, AFT.Identity, AFT.Copy, AFT.Sin, AFT.Relu}

    def _doctored_gat(arch):
        tabs = dict(_real_gat(arch))
        return {n: (set(f) | ours if n == "exp_and_friends" else set(f) - ours)
                for n, f in tabs.items()}

    bacc_mod.get_activation_tables = _doctored_gat

    f32 = mybir.dt.float32
    f16 = mybir.dt.float16
    nc = bacc_mod.Bacc(None, target_bir_lowering=False, debug=False)
    x_d = nc.declare_dram_parameter("x", [N_GROUP, 20, GROUP, FD], f16, isOutput=False)
    win_d = nc.declare_dram_parameter("w_in", [20, ROWS], f16, isOutput=False)
    wh_d = nc.declare_dram_parameter("w_hid", [N_HIDDEN, ROWS, ROWS], f16, isOutput=False)
    wo_d = nc.declare_dram_parameter("w_out", [ROWS, 15], f16, isOutput=False)
    b_d = nc.declare_dram_parameter("bias", [ROWS, 12], f32, isOutput=False)
    bs_d = nc.declare_dram_parameter("sbias", [5, 12], f32, isOutput=False)
    ob_d = nc.declare_dram_parameter("obias", [111, 1], f32, isOutput=False)
    o_d = nc.declare_dram_parameter("out", [N_GROUP, 111, FD], f16, isOutput=True)

    with tile.TileContext(nc) as tc:
        with (tc.tile_pool(name="wpool", bufs=1) as wpool,
              tc.tile_pool(name="xpool", bufs=4) as xpool,
              tc.tile_pool(name="hpool", bufs=2) as hpool,
              tc.tile_pool(name="tpool", bufs=2) as tpool,
              tc.tile_pool(name="opool", bufs=3) as opool,
              tc.tile_pool(name="ppool", bufs=2, space="PSUM") as ppool):
            win = wpool.tile([20, ROWS], f16)
            wh = [wpool.tile([ROWS, ROWS], f16, tag=f"wh{i}", name=f"wh{i}")
                  for i in range(N_HIDDEN)]
            wo = wpool.tile([ROWS, 15], f16)
            bt = wpool.tile([ROWS, 12], f32)
            bs = wpool.tile([5, 12], f32)
            ob = wpool.tile([111, 1], f32)
            nc.sync.dma_start(out=win[:], in_=win_d[:])
            for i in range(N_HIDDEN):
                nc.sync.dma_start(out=wh[i][:], in_=wh_d[i])
            nc.sync.dma_start(out=wo[:], in_=wo_d[:])
            nc.sync.dma_start(out=bt[:], in_=b_d[:])
            nc.sync.dma_start(out=bs[:], in_=bs_d[:])
            nc.sync.dma_start(out=ob[:], in_=ob_d[:])

            HC = GROUP // 2                   # copyB split point (DVE|ACT)
            for pr in range(N_GROUP // 2):
                gpair = (2 * pr, 2 * pr + 1)
                xg = {}
                for par, g in enumerate(gpair):
                    xg[par] = xpool.tile([20, GROUP, FD], f16, tag="xg",
                                         name=f"xg{par}")
                    nc.sync.dma_start(out=xg[par][:], in_=x_d[g])
                hprev = {0: None, 1: None}
                for mm in range(1, 13):       # 12 hidden matmul rounds
                    for par in range(2):
                        P = ppool.tile([ROWS, GROUP, FD], f32, tag="pm")
                        for r in range(GROUP):
                            if mm == 1:
                                nc.tensor.matmul(P[:, r, :], win[:],
                                                 xg[par][:, r, :],
                                                 start=True, stop=True)
                            else:
                                nc.tensor.matmul(P[:, r, :], wh[mm - 2][:],
                                                 hprev[par][:, r, :],
                                                 start=True, stop=True)
                        H = hpool.tile([ROWS, GROUP, FD], f16, tag=f"H{par}")
                        s1 = tpool.tile([5, GROUP, FD], f16, tag=f"s1{par}")
                        s2 = tpool.tile([5, GROUP, FD], f16, tag=f"s2{par}")
                        q = tpool.tile([5, GROUP, FD], f16, tag=f"q{par}")
                        SQ = tpool.tile([20, GROUP, FD], f16, tag=f"sq{par}")
                        # half-group granular chains: each half depends on
                        # only 2 of the 4 matmuls, so elementwise work starts
                        # early and the two halves pipeline across engines.
                        for hh in range(2):
                            rs = slice(2 * hh, 2 * hh + 2)
                            # copyB: H = P + bias (DVE half 0, ACT half 1)
                            if hh == 0:
                                nc.vector.tensor_scalar_add(
                                    H[:, rs, :], P[:, rs, :], bt[:, mm - 1:mm])
                            else:
                                nc.scalar.activation(
                                    H[:, rs, :], P[:, rs, :], AFT.Identity,
                                    bias=bt[:, mm - 1:mm], scale=1.0)
                            # s1 = Sin2pi(P_s/6pi + b') straight from PSUM
                            nc.scalar.activation(s1[:, rs, :],
                                                 P[SIN0:SIN0 + 5, rs, :],
                                                 AFT.Sin, bias=bs[:, mm - 1:mm],
                                                 scale=1.0)
                            # gauss square (needs only this half's copy)
                            nc.vector.tensor_tensor(SQ[:, rs, :],
                                                    H[GA0:GA0 + 20, rs, :],
                                                    H[GA0:GA0 + 20, rs, :],
                                                    op=ALU.mult)
                            # sin polish: H_s = s1*(3-4*s1^2)
                            nc.vector.tensor_tensor(s2[:, rs, :], s1[:, rs, :],
                                                    s1[:, rs, :], op=ALU.mult)
                            nc.vector.tensor_scalar(q[:, rs, :], s2[:, rs, :],
                                                    -4.0, 3.0,
                                                    op0=ALU.mult, op1=ALU.add)
                            nc.vector.tensor_tensor(H[SIN0:SIN0 + 5, rs, :],
                                                    q[:, rs, :], s1[:, rs, :],
                                                    op=ALU.mult)
                            nc.scalar.activation(H[GA0:GA0 + 20, rs, :],
                                                 SQ[:, rs, :],
                                                 AFT.Exp, bias=0.0, scale=-1.0)
                        hprev[par] = H
                for par, g in enumerate(gpair):   # output round
                    O = ppool.tile([111, FD], f32, tag="pm")
                    for r in range(GROUP):
                        nc.tensor.matmul(O[32 * r:32 * r + 15, :], wo[:],
                                         hprev[par][:, r, :], start=True,
                                         stop=True, tile_position=(0, 32 * r))
                    ot = opool.tile([111, FD], f16, tag="ot")
                    nc.vector.tensor_scalar_add(ot[:], O[:], ob[:])
                    nc.sync.dma_start(out=o_d[g], in_=ot[:])
    nc.compile()

    _orig = nc.to_json_bytes
    nc.to_json_bytes = lambda: _orig().replace(b'"func":"Sin"', b'"func":"Sin2pi"')
    return nc

def _get_nc():
    if "nc" not in _CACHE:
        _CACHE["nc"] = _build()
    return _CACHE["nc"]

def make_in_maps(x_cores, w):
    return [{"x": x_cores[k], "w_in": w["w_in"], "w_hid": w["w_hid"],
             "w_out": w["w_out"], "bias": w["bias"], "sbias": w["sbias"],
             "obias": w["obias"]} for k in range(N_CORES)]

def run_device(x_cores, w):
    from concourse.bass_utils import run_bass_kernel_spmd
    nc = _get_nc()
    res = run_bass_kernel_spmd(nc, make_in_maps(x_cores, w),
                               list(range(N_CORES)), trace=False)
    return [res.results[k]["out"] for k in range(N_CORES)]

def kernel(x, W_in, W_hidden, W_out):
    w = pack_weights(W_in, W_hidden, W_out)
    x_cores = pack_x(x)
    outs = run_device(x_cores, w)
    return unpack_out(outs)
